# revision 7
# baseline (speedup 1.0000x reference)
"""Trainium2 Bass kernel for nn_Attention_86646670230179 (eager MHA, f32 I/O).

Strategy (8 NeuronCores, tensor-parallel over heads):
  - Each core owns 2 of the 16 heads (a 128-wide slice of the internal dim).
  - Host stages q/k/v transposed ([E, B*L], bf16) so every matmul contracts
    along the SBUF partition dim. Score scale (1/8) is folded into Wq.
  - Per core: qp^T/kp^T projections (transposed layout), vp projection
    (natural layout via on-chip PE transpose of vp^T), scores^T = kh^T.T @ qh^T
    with both heads packed into the 128-row PE array (K=64 each),
    exp on ScalarE (no max subtraction: scores ~ N(0,1)), PV matmul with an
    appended ones-column producing unnormalized outputs + row sums in one
    accumulation, normalization via a K=1 broadcast matmul + DVE multiply.
  - Per-head outputs (outh^T, bf16) are AllGathered (one collective per batch)
    and every core computes a 128-wide column slice of the output projection
    in transposed orientation (out^T = Wo_c^T @ outh^T, N=512 matmuls).
  - Host concatenates the 8 column slices, transposes, and adds the bias
    terms (bv @ Wo + bo), which commute with attention exactly because
    softmax rows sum to 1.
"""
import sys
from contextlib import ExitStack

import numpy as np

sys.path.insert(0, "/opt/trn_rl_repo")

import ml_dtypes  # noqa: E402
import concourse.bass as bass  # noqa: E402
import concourse.mybir as mybir  # noqa: E402
import concourse.tile as tile  # noqa: E402
from concourse import bacc  # noqa: E402
from concourse.bass_utils import run_bass_kernel_spmd  # noqa: E402
from concourse.masks import make_identity  # noqa: E402

BF16 = mybir.dt.bfloat16
F32 = mybir.dt.float32
AF = mybir.ActivationFunctionType

NCORES = 8
B, L, E, H = 2, 2048, 1024, 16
S = L
D = E // H            # 64 head dim
R = B * L             # 4096 total rows
HC = H // NCORES      # 2 heads per core
EC = HC * D           # 128 channel slice per core
KT = E // 128         # 8 contraction tiles
NT = L // 512         # 4 512-wide row tiles per batch
ST = S // 128         # 16 key tiles per batch
DP1 = D + 1           # 65: head dim + ones column


def build_nc():
    nc = bacc.Bacc("TRN2", target_bir_lowering=False, num_devices=NCORES)

    qT = nc.declare_dram_parameter("qT", [E, R], BF16, isOutput=False)
    kT = nc.declare_dram_parameter("kT", [E, R], BF16, isOutput=False)
    vT = nc.declare_dram_parameter("vT", [E, R], BF16, isOutput=False)
    wq = nc.declare_dram_parameter("wq", [E, EC], BF16, isOutput=False)
    wk = nc.declare_dram_parameter("wk", [E, EC], BF16, isOutput=False)
    wv = nc.declare_dram_parameter("wv", [E, EC], BF16, isOutput=False)
    wo = nc.declare_dram_parameter("wo", [E, EC], BF16, isOutput=False)
    bq = nc.declare_dram_parameter("bq", [EC, 1], F32, isOutput=False)
    bk = nc.declare_dram_parameter("bk", [EC, 1], F32, isOutput=False)
    outT = nc.declare_dram_parameter("outT", [EC, R], F32, isOutput=True)

    with tile.TileContext(nc) as tc, ExitStack() as ctx:
        dram = ctx.enter_context(tc.tile_pool(name="dram", bufs=1, space="DRAM"))
        consts = ctx.enter_context(tc.tile_pool(name="consts", bufs=1))
        xt_pool = ctx.enter_context(tc.tile_pool(name="xt", bufs=9))
        vpt_pool = ctx.enter_context(tc.tile_pool(name="vpt", bufs=2))
        exp_pool = ctx.enter_context(tc.tile_pool(name="expp", bufs=3))
        ot_pool = ctx.enter_context(tc.tile_pool(name="otp", bufs=3))
        ag_pool = ctx.enter_context(tc.tile_pool(name="agp", bufs=2))
        ov_pool = ctx.enter_context(tc.tile_pool(name="ovp", bufs=2))
        rc_pool = ctx.enter_context(tc.tile_pool(name="rcp", bufs=3))
        # PSUM: sc 2x[128,1024] (4 banks) + pv 3x[128,512] (3) + pp 1x[128,512] (1)
        psum_sc = ctx.enter_context(tc.tile_pool(name="psc", bufs=2, space="PSUM"))
        psum_pv = ctx.enter_context(tc.tile_pool(name="ppv", bufs=3, space="PSUM"))
        psum_pp = ctx.enter_context(tc.tile_pool(name="ppp", bufs=1, space="PSUM"))

        # ---- constants / weights staging
        wq_sb = consts.tile([128, KT, EC], BF16, tag="wq")
        wk_sb = consts.tile([128, KT, EC], BF16, tag="wk")
        wv_sb = consts.tile([128, KT, EC], BF16, tag="wv")
        wo_sb = consts.tile([128, KT, EC], BF16, tag="wo")
        for w_sb, w in ((wq_sb, wq), (wk_sb, wk), (wv_sb, wv), (wo_sb, wo)):
            nc.sync.dma_start(w_sb[:], w[:].rearrange("(ko p) m -> p ko m", p=128))
        bq_sb = consts.tile([EC, 1], F32, tag="bq")
        bk_sb = consts.tile([EC, 1], F32, tag="bk")
        nc.sync.dma_start(bq_sb[:], bq[:])
        nc.sync.dma_start(bk_sb[:], bk[:])
        ones_sb = consts.tile([1, D], BF16, tag="ones")
        nc.vector.memset(ones_sb[:], 1.0)
        ident = consts.tile([128, 128], BF16, tag="ident")
        make_identity(nc, ident[:])

        # persistent per-batch activations
        qpT = [consts.tile([128, L], BF16, tag=f"qpT{b}", name=f"qpT{b}") for b in range(B)]
        kpT = [consts.tile([128, L], BF16, tag=f"kpT{b}", name=f"kpT{b}") for b in range(B)]
        vp = [consts.tile([128, ST, 2 * DP1], BF16, tag=f"vp{b}", name=f"vp{b}") for b in range(B)]
        for b in range(B):
            nc.vector.memset(vp[b][:, :, D], 1.0)
            nc.vector.memset(vp[b][:, :, 2 * D + 1], 1.0)

        outhT = [dram.tile([EC, L], BF16, tag=f"outhT{b}", name=f"outhT{b}") for b in range(B)]
        ag = [dram.tile([EC * NCORES, L], BF16, tag=f"ag{b}", name=f"ag{b}", addr_space="Shared") for b in range(B)]

        def project(b):
            """qp^T, kp^T (transposed) and vp (natural) for batch b."""
            for name, xsrc, w_sb, bias, dstT in (
                ("q", qT, wq_sb, bq_sb, qpT[b]),
                ("k", kT, wk_sb, bk_sb, kpT[b]),
                ("v", vT, wv_sb, None, None),
            ):
                xts = []
                for kt in range(KT):
                    xt = xt_pool.tile([128, L], BF16, tag="xt")
                    nc.sync.dma_start(
                        xt[:], xsrc[kt * 128:(kt + 1) * 128, b * L:(b + 1) * L]
                    )
                    xts.append(xt)
                for n in range(NT):
                    ps = psum_pp.tile([128, 512], F32, tag="pp")
                    for kt in range(KT):
                        nc.tensor.matmul(
                            ps[:],
                            lhsT=w_sb[:, kt, :],
                            rhs=xts[kt][:, n * 512:(n + 1) * 512],
                            start=(kt == 0),
                            stop=(kt == KT - 1),
                        )
                    if name != "v":
                        # evacuate with per-partition bias add, cast to bf16
                        nc.vector.tensor_tensor(
                            dstT[:, n * 512:(n + 1) * 512], ps[:],
                            bias[:].to_broadcast((EC, 512)),
                            mybir.AluOpType.add,
                        )
                    else:
                        vpt = vpt_pool.tile([128, 512], BF16, tag="vpt")
                        nc.vector.tensor_copy(vpt[:], ps[:])
                        for mb in range(4):
                            trp = psum_pp.tile([128, 128], BF16, tag="pp")
                            nc.tensor.transpose(
                                trp[:], vpt[:, mb * 128:(mb + 1) * 128], ident[:]
                            )
                            st_idx = n * 4 + mb
                            nc.vector.tensor_copy(
                                vp[b][:, st_idx, 0:D], trp[:, 0:D]
                            )
                            nc.vector.tensor_copy(
                                vp[b][:, st_idx, DP1:DP1 + D], trp[:, D:2 * D]
                            )

        def attention(b):
            for lt in range(NT):
                po = []
                for h in range(HC):
                    p = psum_pv.tile([128, 512], F32, tag="pv", name=f"po{h}")
                    po.append(p)
                for st in range(ST):
                    ps = psum_sc.tile([128, 1024], F32, tag="sc")
                    for h in range(HC):
                        nc.tensor.matmul(
                            ps[:, h * 512:(h + 1) * 512],
                            lhsT=kpT[b][h * D:(h + 1) * D, st * 128:(st + 1) * 128],
                            rhs=qpT[b][h * D:(h + 1) * D, lt * 512:(lt + 1) * 512],
                            start=True,
                            stop=True,
                            tile_position=(h * D, 0),
                        )
                    ex = exp_pool.tile([128, 1024], BF16, tag="exp")
                    nc.scalar.activation(ex[:], ps[:], AF.Exp)
                    for h in range(HC):
                        nc.tensor.matmul(
                            po[h][0:DP1, :],
                            lhsT=vp[b][:, st, h * DP1:(h + 1) * DP1],
                            rhs=ex[:, h * 512:(h + 1) * 512],
                            start=(st == 0),
                            stop=(st == ST - 1),
                        )
                for h in range(HC):
                    rc = rc_pool.tile([1, 512], F32, tag="rc")
                    rc16 = rc_pool.tile([1, 512], BF16, tag="rc16")
                    nc.vector.reciprocal(rc[:], po[h][D:DP1, :])
                    nc.vector.tensor_copy(rc16[:], rc[:])
                    pb = psum_pv.tile([128, 512], F32, tag="pv")
                    nc.tensor.matmul(
                        pb[0:D, :], lhsT=ones_sb[:], rhs=rc16[:],
                        start=True, stop=True,
                    )
                    pb_sb = ot_pool.tile([D, 512], BF16, tag="pbsb")
                    nc.vector.tensor_copy(pb_sb[:], pb[0:D, :])
                    ot = ot_pool.tile([D, 512], BF16, tag="ot")
                    nc.vector.tensor_mul(ot[:], po[h][0:D, :], pb_sb[:])
                    nc.sync.dma_start(
                        outhT[b][h * D:(h + 1) * D, lt * 512:(lt + 1) * 512], ot[:]
                    )

        def gather_and_project_out(b):
            nc.gpsimd.collective_compute(
                "AllGather",
                mybir.AluOpType.bypass,
                replica_groups=[list(range(NCORES))],
                ins=[outhT[b][:]],
                outs=[ag[b][:]],
            )
            ag_sb = ag_pool.tile([128, KT, L], BF16, tag="ag")
            nc.sync.dma_start(
                ag_sb[:], ag[b][:].rearrange("(c p) r -> p c r", p=128)
            )
            for n in range(NT):
                pt = psum_pp.tile([128, 512], F32, tag="pp")
                for c2 in range(KT):
                    nc.tensor.matmul(
                        pt[:],
                        lhsT=wo_sb[:, c2, :],
                        rhs=ag_sb[:, c2, n * 512:(n + 1) * 512],
                        start=(c2 == 0),
                        stop=(c2 == KT - 1),
                    )
                ov = ov_pool.tile([128, 512], F32, tag="ov")
                nc.vector.tensor_copy(ov[:], pt[:])
                nc.sync.dma_start(
                    outT[:, b * L + n * 512: b * L + (n + 1) * 512], ov[:]
                )

        project(0)
        attention(0)
        gather_and_project_out(0)
        project(1)
        attention(1)
        gather_and_project_out(1)

    nc.compile()
    return nc


_NC_CACHE = {}


def _get_nc():
    if "nc" not in _NC_CACHE:
        _NC_CACHE["nc"] = build_nc()
    return _NC_CACHE["nc"]


def kernel(q, k, v, Wq, bq, Wk, bk, Wv, bv, Wo, bo, _trace=False, _tmpdir=None):
    bf = ml_dtypes.bfloat16
    scale = np.float32(1.0 / np.sqrt(D))  # 0.125, exact

    q2 = np.asarray(q, np.float32).reshape(R, E)
    k2 = np.asarray(k, np.float32).reshape(R, E)
    v2 = np.asarray(v, np.float32).reshape(R, E)
    qTh = np.ascontiguousarray(q2.T).astype(bf)
    kTh = np.ascontiguousarray(k2.T).astype(bf)
    vTh = np.ascontiguousarray(v2.T).astype(bf)
    Wq = np.asarray(Wq, np.float32)
    Wk = np.asarray(Wk, np.float32)
    Wv = np.asarray(Wv, np.float32)
    Wo = np.asarray(Wo, np.float32)

    in_maps = []
    for c in range(NCORES):
        sl = slice(c * EC, (c + 1) * EC)
        in_maps.append({
            "qT": qTh,
            "kT": kTh,
            "vT": vTh,
            "wq": np.ascontiguousarray(Wq[:, sl] * scale).astype(bf),
            "wk": np.ascontiguousarray(Wk[:, sl]).astype(bf),
            "wv": np.ascontiguousarray(Wv[:, sl]).astype(bf),
            "wo": np.ascontiguousarray(Wo[:, sl]).astype(bf),
            "bq": (np.asarray(bq, np.float32)[sl] * scale).reshape(EC, 1).copy(),
            "bk": np.asarray(bk, np.float32)[sl].reshape(EC, 1).copy(),
        })

    nc = _get_nc()
    res = run_bass_kernel_spmd(
        nc, in_maps, list(range(NCORES)), trace=_trace, tmpdir=_tmpdir
    )
    outT_full = np.concatenate(
        [np.asarray(res.results[c]["outT"], np.float32) for c in range(NCORES)],
        axis=0,
    )  # [E, R]
    out = np.ascontiguousarray(outT_full.T)  # [R, E]
    # bv passes through attention unchanged (softmax rows sum to 1):
    # out += bv @ Wo + bo
    host_bias = (
        np.asarray(bv, np.float64) @ np.asarray(Wo, np.float64)
        + np.asarray(bo, np.float64)
    ).astype(np.float32)
    out += host_bias[None, :]
    if _trace:
        return out.reshape(B, L, E), res
    return out.reshape(B, L, E)


# revision 8
# speedup vs baseline: 1.0110x; 1.0110x over previous
"""Trainium2 Bass kernel for nn_Attention_86646670230179 (eager MHA, f32 I/O).

Strategy (8 NeuronCores, tensor-parallel over heads):
  - Each core owns 2 of the 16 heads (a 128-wide slice of the internal dim).
  - Host stages q/k/v transposed ([E, B*L], bf16) so every matmul contracts
    along the SBUF partition dim. Score scale (1/8) is folded into Wq.
  - Per core: qp^T/kp^T projections (transposed layout), vp projection
    (natural layout via on-chip PE transpose of vp^T), scores^T = kh^T.T @ qh^T
    with both heads packed into the 128-row PE array (K=64 each),
    exp on ScalarE (no max subtraction: scores ~ N(0,1)), PV matmul with an
    appended ones-column producing unnormalized outputs + row sums in one
    accumulation, normalization via a K=1 broadcast matmul + DVE multiply.
  - Per-head outputs (outh^T, bf16) are AllGathered (one collective per batch)
    and every core computes a 128-wide column slice of the output projection
    in transposed orientation (out^T = Wo_c^T @ outh^T, N=512 matmuls).
  - Host concatenates the 8 column slices, transposes, and adds the bias
    terms (bv @ Wo + bo), which commute with attention exactly because
    softmax rows sum to 1.
"""
import sys
from contextlib import ExitStack

import numpy as np

sys.path.insert(0, "/opt/trn_rl_repo")

import ml_dtypes  # noqa: E402
import concourse.bass as bass  # noqa: E402
import concourse.mybir as mybir  # noqa: E402
import concourse.tile as tile  # noqa: E402
from concourse import bacc  # noqa: E402
from concourse.bass_utils import run_bass_kernel_spmd  # noqa: E402
from concourse.masks import make_identity  # noqa: E402

BF16 = mybir.dt.bfloat16
F32 = mybir.dt.float32
AF = mybir.ActivationFunctionType

NCORES = 8
B, L, E, H = 2, 2048, 1024, 16
S = L
D = E // H            # 64 head dim
R = B * L             # 4096 total rows
HC = H // NCORES      # 2 heads per core
EC = HC * D           # 128 channel slice per core
KT = E // 128         # 8 contraction tiles
NT = L // 512         # 4 512-wide row tiles per batch
ST = S // 128         # 16 key tiles per batch
DP1 = D + 1           # 65: head dim + ones column


def build_nc():
    nc = bacc.Bacc("TRN2", target_bir_lowering=False, num_devices=NCORES)

    qT = nc.declare_dram_parameter("qT", [E, R], BF16, isOutput=False)
    kT = nc.declare_dram_parameter("kT", [E, R], BF16, isOutput=False)
    vT = nc.declare_dram_parameter("vT", [E, R], BF16, isOutput=False)
    wq = nc.declare_dram_parameter("wq", [E, EC], BF16, isOutput=False)
    wk = nc.declare_dram_parameter("wk", [E, EC], BF16, isOutput=False)
    wv = nc.declare_dram_parameter("wv", [E, EC], BF16, isOutput=False)
    wo = nc.declare_dram_parameter("wo", [E, EC], BF16, isOutput=False)
    bq = nc.declare_dram_parameter("bq", [EC, 1], F32, isOutput=False)
    bk = nc.declare_dram_parameter("bk", [EC, 1], F32, isOutput=False)
    outT = nc.declare_dram_parameter("outT", [EC, R], F32, isOutput=True)

    with tile.TileContext(nc) as tc, ExitStack() as ctx:
        dram = ctx.enter_context(tc.tile_pool(name="dram", bufs=1, space="DRAM"))
        consts = ctx.enter_context(tc.tile_pool(name="consts", bufs=1))
        xt_pool = ctx.enter_context(tc.tile_pool(name="xt", bufs=9))
        vpt_pool = ctx.enter_context(tc.tile_pool(name="vpt", bufs=2))
        exp_pool = ctx.enter_context(tc.tile_pool(name="expp", bufs=3))
        ot_pool = ctx.enter_context(tc.tile_pool(name="otp", bufs=3))
        ag_pool = ctx.enter_context(tc.tile_pool(name="agp", bufs=2))
        ov_pool = ctx.enter_context(tc.tile_pool(name="ovp", bufs=2))
        rc_pool = ctx.enter_context(tc.tile_pool(name="rcp", bufs=3))
        # PSUM: sc 2x[128,1024] (4 banks) + pv 3x[128,512] (3) + pp 1x[128,512] (1)
        psum_sc = ctx.enter_context(tc.tile_pool(name="psc", bufs=2, space="PSUM"))
        psum_pv = ctx.enter_context(tc.tile_pool(name="ppv", bufs=3, space="PSUM"))
        psum_pp = ctx.enter_context(tc.tile_pool(name="ppp", bufs=1, space="PSUM"))

        # ---- constants / weights staging
        wq_sb = consts.tile([128, KT, EC], BF16, tag="wq")
        wk_sb = consts.tile([128, KT, EC], BF16, tag="wk")
        wv_sb = consts.tile([128, KT, EC], BF16, tag="wv")
        wo_sb = consts.tile([128, KT, EC], BF16, tag="wo")
        for w_sb, w in ((wq_sb, wq), (wk_sb, wk), (wv_sb, wv), (wo_sb, wo)):
            nc.sync.dma_start(w_sb[:], w[:].rearrange("(ko p) m -> p ko m", p=128))
        bq_sb = consts.tile([EC, 1], F32, tag="bq")
        bk_sb = consts.tile([EC, 1], F32, tag="bk")
        nc.sync.dma_start(bq_sb[:], bq[:])
        nc.sync.dma_start(bk_sb[:], bk[:])
        ones_sb = consts.tile([1, D], BF16, tag="ones")
        nc.vector.memset(ones_sb[:], 1.0)
        ident = consts.tile([128, 128], BF16, tag="ident")
        make_identity(nc, ident[:])

        # persistent per-batch activations
        qpT = [consts.tile([128, L], BF16, tag=f"qpT{b}", name=f"qpT{b}") for b in range(B)]
        kpT = [consts.tile([128, L], BF16, tag=f"kpT{b}", name=f"kpT{b}") for b in range(B)]
        vp = [consts.tile([128, ST, 2 * DP1], BF16, tag=f"vp{b}", name=f"vp{b}") for b in range(B)]
        for b in range(B):
            nc.vector.memset(vp[b][:, :, D], 1.0)
            nc.vector.memset(vp[b][:, :, 2 * D + 1], 1.0)

        outhT = [dram.tile([EC, L], BF16, tag=f"outhT{b}", name=f"outhT{b}") for b in range(B)]
        ag = [dram.tile([EC * NCORES, L], BF16, tag=f"ag{b}", name=f"ag{b}", addr_space="Shared") for b in range(B)]

        def project(b):
            """qp^T, kp^T (transposed) and vp (natural) for batch b."""
            for name, xsrc, w_sb, bias, dstT in (
                ("k", kT, wk_sb, bk_sb, kpT[b]),
                ("v", vT, wv_sb, None, None),
                ("q", qT, wq_sb, bq_sb, qpT[b]),
            ):
                xts = []
                for kt in range(KT):
                    xt = xt_pool.tile([128, L], BF16, tag="xt")
                    nc.sync.dma_start(
                        xt[:], xsrc[kt * 128:(kt + 1) * 128, b * L:(b + 1) * L]
                    )
                    xts.append(xt)
                for n in range(NT):
                    ps = psum_pp.tile([128, 512], F32, tag="pp")
                    for kt in range(KT):
                        nc.tensor.matmul(
                            ps[:],
                            lhsT=w_sb[:, kt, :],
                            rhs=xts[kt][:, n * 512:(n + 1) * 512],
                            start=(kt == 0),
                            stop=(kt == KT - 1),
                        )
                    if name != "v":
                        # evacuate with per-partition bias add, cast to bf16
                        nc.vector.tensor_tensor(
                            dstT[:, n * 512:(n + 1) * 512], ps[:],
                            bias[:].to_broadcast((EC, 512)),
                            mybir.AluOpType.add,
                        )
                    else:
                        vpt = vpt_pool.tile([128, 512], BF16, tag="vpt")
                        nc.vector.tensor_copy(vpt[:], ps[:])
                        for mb in range(4):
                            trp = psum_pp.tile([128, 128], BF16, tag="pp")
                            nc.tensor.transpose(
                                trp[:], vpt[:, mb * 128:(mb + 1) * 128], ident[:]
                            )
                            st_idx = n * 4 + mb
                            nc.vector.tensor_copy(
                                vp[b][:, st_idx, 0:D], trp[:, 0:D]
                            )
                            nc.vector.tensor_copy(
                                vp[b][:, st_idx, DP1:DP1 + D], trp[:, D:2 * D]
                            )

        def attention(b):
            for lt in range(NT):
                po = []
                for h in range(HC):
                    p = psum_pv.tile([128, 512], F32, tag="pv", name=f"po{h}")
                    po.append(p)
                for st in range(ST):
                    ps = psum_sc.tile([128, 1024], F32, tag="sc")
                    for h in range(HC):
                        nc.tensor.matmul(
                            ps[:, h * 512:(h + 1) * 512],
                            lhsT=kpT[b][h * D:(h + 1) * D, st * 128:(st + 1) * 128],
                            rhs=qpT[b][h * D:(h + 1) * D, lt * 512:(lt + 1) * 512],
                            start=True,
                            stop=True,
                            tile_position=(h * D, 0),
                        )
                    ex = exp_pool.tile([128, 1024], BF16, tag="exp")
                    nc.scalar.activation(ex[:], ps[:], AF.Exp)
                    for h in range(HC):
                        nc.tensor.matmul(
                            po[h][0:DP1, :],
                            lhsT=vp[b][:, st, h * DP1:(h + 1) * DP1],
                            rhs=ex[:, h * 512:(h + 1) * 512],
                            start=(st == 0),
                            stop=(st == ST - 1),
                        )
                for h in range(HC):
                    rc = rc_pool.tile([1, 512], F32, tag="rc")
                    rc16 = rc_pool.tile([1, 512], BF16, tag="rc16")
                    nc.vector.reciprocal(rc[:], po[h][D:DP1, :])
                    nc.vector.tensor_copy(rc16[:], rc[:])
                    pb = psum_pp.tile([128, 512], F32, tag="pp", name="pb")
                    nc.tensor.matmul(
                        pb[0:D, :], lhsT=ones_sb[:], rhs=rc16[:],
                        start=True, stop=True,
                    )
                    pb_sb = ot_pool.tile([D, 512], BF16, tag="pbsb")
                    nc.vector.tensor_copy(pb_sb[:], pb[0:D, :])
                    ot = ot_pool.tile([D, 512], BF16, tag="ot")
                    nc.vector.tensor_mul(ot[:], po[h][0:D, :], pb_sb[:])
                    nc.sync.dma_start(
                        outhT[b][h * D:(h + 1) * D, lt * 512:(lt + 1) * 512], ot[:]
                    )

        def gather(b):
            nc.gpsimd.collective_compute(
                "AllGather",
                mybir.AluOpType.bypass,
                replica_groups=[list(range(NCORES))],
                ins=[outhT[b][:]],
                outs=[ag[b][:]],
            )

        def project_out(b):
            ag_sb = ag_pool.tile([128, KT, L], BF16, tag="ag")
            nc.sync.dma_start(
                ag_sb[:], ag[b][:].rearrange("(c p) r -> p c r", p=128)
            )
            for n in range(NT):
                pt = psum_pp.tile([128, 512], F32, tag="pp")
                for c2 in range(KT):
                    nc.tensor.matmul(
                        pt[:],
                        lhsT=wo_sb[:, c2, :],
                        rhs=ag_sb[:, c2, n * 512:(n + 1) * 512],
                        start=(c2 == 0),
                        stop=(c2 == KT - 1),
                    )
                ov = ov_pool.tile([128, 512], F32, tag="ov")
                nc.vector.tensor_copy(ov[:], pt[:])
                nc.sync.dma_start(
                    outT[:, b * L + n * 512: b * L + (n + 1) * 512], ov[:]
                )

        project(0)
        attention(0)
        gather(0)
        project(1)
        project_out(0)
        attention(1)
        gather(1)
        project_out(1)

    nc.compile()
    return nc


_NC_CACHE = {}


def _get_nc():
    if "nc" not in _NC_CACHE:
        _NC_CACHE["nc"] = build_nc()
    return _NC_CACHE["nc"]


def kernel(q, k, v, Wq, bq, Wk, bk, Wv, bv, Wo, bo, _trace=False, _tmpdir=None):
    bf = ml_dtypes.bfloat16
    scale = np.float32(1.0 / np.sqrt(D))  # 0.125, exact

    q2 = np.asarray(q, np.float32).reshape(R, E)
    k2 = np.asarray(k, np.float32).reshape(R, E)
    v2 = np.asarray(v, np.float32).reshape(R, E)
    qTh = np.ascontiguousarray(q2.T).astype(bf)
    kTh = np.ascontiguousarray(k2.T).astype(bf)
    vTh = np.ascontiguousarray(v2.T).astype(bf)
    Wq = np.asarray(Wq, np.float32)
    Wk = np.asarray(Wk, np.float32)
    Wv = np.asarray(Wv, np.float32)
    Wo = np.asarray(Wo, np.float32)

    in_maps = []
    for c in range(NCORES):
        sl = slice(c * EC, (c + 1) * EC)
        in_maps.append({
            "qT": qTh,
            "kT": kTh,
            "vT": vTh,
            "wq": np.ascontiguousarray(Wq[:, sl] * scale).astype(bf),
            "wk": np.ascontiguousarray(Wk[:, sl]).astype(bf),
            "wv": np.ascontiguousarray(Wv[:, sl]).astype(bf),
            "wo": np.ascontiguousarray(Wo[:, sl]).astype(bf),
            "bq": (np.asarray(bq, np.float32)[sl] * scale).reshape(EC, 1).copy(),
            "bk": np.asarray(bk, np.float32)[sl].reshape(EC, 1).copy(),
        })

    nc = _get_nc()
    res = run_bass_kernel_spmd(
        nc, in_maps, list(range(NCORES)), trace=_trace, tmpdir=_tmpdir
    )
    outT_full = np.concatenate(
        [np.asarray(res.results[c]["outT"], np.float32) for c in range(NCORES)],
        axis=0,
    )  # [E, R]
    out = np.ascontiguousarray(outT_full.T)  # [R, E]
    # bv passes through attention unchanged (softmax rows sum to 1):
    # out += bv @ Wo + bo
    host_bias = (
        np.asarray(bv, np.float64) @ np.asarray(Wo, np.float64)
        + np.asarray(bo, np.float64)
    ).astype(np.float32)
    out += host_bias[None, :]
    if _trace:
        return out.reshape(B, L, E), res
    return out.reshape(B, L, E)


# revision 9
# speedup vs baseline: 1.0370x; 1.0257x over previous
"""Trainium2 Bass kernel for nn_Attention_86646670230179 (eager MHA, f32 I/O).

Strategy (8 NeuronCores, tensor-parallel over heads):
  - Each core owns 2 of the 16 heads (a 128-wide slice of the internal dim).
  - Host stages q/k/v transposed ([E, B*L], bf16) so every matmul contracts
    along the SBUF partition dim. Score scale (1/8) is folded into Wq.
  - Per core: qp^T/kp^T projections (transposed layout), vp projection
    (natural layout via on-chip PE transpose of vp^T), scores^T = kh^T.T @ qh^T
    with both heads packed into the 128-row PE array (K=64 each),
    exp on ScalarE (no max subtraction: scores ~ N(0,1)), PV matmul with an
    appended ones-column producing unnormalized outputs + row sums in one
    accumulation, normalization via a K=1 broadcast matmul + DVE multiply.
  - Per-head outputs (outh^T, bf16) are AllGathered (one collective per batch)
    and every core computes a 128-wide column slice of the output projection
    in transposed orientation (out^T = Wo_c^T @ outh^T, N=512 matmuls).
  - Host concatenates the 8 column slices, transposes, and adds the bias
    terms (bv @ Wo + bo), which commute with attention exactly because
    softmax rows sum to 1.
"""
import sys
from contextlib import ExitStack

import numpy as np

sys.path.insert(0, "/opt/trn_rl_repo")

import ml_dtypes  # noqa: E402
import concourse.bass as bass  # noqa: E402
import concourse.mybir as mybir  # noqa: E402
import concourse.tile as tile  # noqa: E402
from concourse import bacc  # noqa: E402
from concourse.bass_utils import run_bass_kernel_spmd  # noqa: E402
from concourse.masks import make_identity  # noqa: E402

BF16 = mybir.dt.bfloat16
F32 = mybir.dt.float32
AF = mybir.ActivationFunctionType

NCORES = 8
B, L, E, H = 2, 2048, 1024, 16
S = L
D = E // H            # 64 head dim
R = B * L             # 4096 total rows
HC = H // NCORES      # 2 heads per core
EC = HC * D           # 128 channel slice per core
KT = E // 128         # 8 contraction tiles
NT = L // 512         # 4 512-wide row tiles per batch
ST = S // 128         # 16 key tiles per batch
DP1 = D + 1           # 65: head dim + ones column


def build_nc():
    nc = bacc.Bacc("TRN2", target_bir_lowering=False, num_devices=NCORES)

    qT = nc.declare_dram_parameter("qT", [E, R], BF16, isOutput=False)
    kT = nc.declare_dram_parameter("kT", [E, R], BF16, isOutput=False)
    vT = nc.declare_dram_parameter("vT", [E, R], BF16, isOutput=False)
    wq = nc.declare_dram_parameter("wq", [E, EC], BF16, isOutput=False)
    wk = nc.declare_dram_parameter("wk", [E, EC], BF16, isOutput=False)
    wv = nc.declare_dram_parameter("wv", [E, EC], BF16, isOutput=False)
    wo = nc.declare_dram_parameter("wo", [E, EC], BF16, isOutput=False)
    bq = nc.declare_dram_parameter("bq", [EC, 1], F32, isOutput=False)
    bk = nc.declare_dram_parameter("bk", [EC, 1], F32, isOutput=False)
    outT = nc.declare_dram_parameter("outT", [EC, R], F32, isOutput=True)

    with tile.TileContext(nc) as tc, ExitStack() as ctx:
        dram = ctx.enter_context(tc.tile_pool(name="dram", bufs=1, space="DRAM"))
        consts = ctx.enter_context(tc.tile_pool(name="consts", bufs=1))
        xt_pool = ctx.enter_context(tc.tile_pool(name="xt", bufs=9))
        vpt_pool = ctx.enter_context(tc.tile_pool(name="vpt", bufs=2))
        exp_pool = ctx.enter_context(tc.tile_pool(name="expp", bufs=3))
        ot_pool = ctx.enter_context(tc.tile_pool(name="otp", bufs=3))
        ag_pool = ctx.enter_context(tc.tile_pool(name="agp", bufs=2))
        ov_pool = ctx.enter_context(tc.tile_pool(name="ovp", bufs=2))
        rc_pool = ctx.enter_context(tc.tile_pool(name="rcp", bufs=3))
        # PSUM: sc 2x[128,1024] (4 banks) + pv 3x[128,512] (3) + pp 1x[128,512] (1)
        psum_sc = ctx.enter_context(tc.tile_pool(name="psc", bufs=2, space="PSUM"))
        psum_pv = ctx.enter_context(tc.tile_pool(name="ppv", bufs=3, space="PSUM"))
        psum_pp = ctx.enter_context(tc.tile_pool(name="ppp", bufs=1, space="PSUM"))

        # ---- constants / weights staging
        wq_sb = consts.tile([128, KT, EC], BF16, tag="wq")
        wk_sb = consts.tile([128, KT, EC], BF16, tag="wk")
        wv_sb = consts.tile([128, KT, EC], BF16, tag="wv")
        wo_sb = consts.tile([128, KT, EC], BF16, tag="wo")
        for w_sb, w in ((wq_sb, wq), (wk_sb, wk), (wv_sb, wv), (wo_sb, wo)):
            nc.sync.dma_start(w_sb[:], w[:].rearrange("(ko p) m -> p ko m", p=128))
        bq_sb = consts.tile([EC, 1], F32, tag="bq")
        bk_sb = consts.tile([EC, 1], F32, tag="bk")
        nc.sync.dma_start(bq_sb[:], bq[:])
        nc.sync.dma_start(bk_sb[:], bk[:])
        ones_sb = consts.tile([1, D], BF16, tag="ones")
        nc.vector.memset(ones_sb[:], 1.0)
        ident = consts.tile([128, 128], BF16, tag="ident")
        make_identity(nc, ident[:])

        # persistent per-batch activations
        qpT = [consts.tile([128, L], BF16, tag=f"qpT{b}", name=f"qpT{b}") for b in range(B)]
        kpT = [consts.tile([128, L], BF16, tag=f"kpT{b}", name=f"kpT{b}") for b in range(B)]
        vp = [consts.tile([128, ST, 2 * DP1], BF16, tag=f"vp{b}", name=f"vp{b}") for b in range(B)]
        for b in range(B):
            nc.vector.memset(vp[b][:, :, D], 1.0)
            nc.vector.memset(vp[b][:, :, 2 * D + 1], 1.0)

        outhT = [dram.tile([EC, L], BF16, tag=f"outhT{b}", name=f"outhT{b}") for b in range(B)]
        ag = [dram.tile([EC * NCORES, L], BF16, tag=f"ag{b}", name=f"ag{b}", addr_space="Shared") for b in range(B)]

        def project(b):
            """qp^T, kp^T (transposed) and vp (natural) for batch b."""
            for name, xsrc, w_sb, bias, dstT in (
                ("k", kT, wk_sb, bk_sb, kpT[b]),
                ("v", vT, wv_sb, None, None),
                ("q", qT, wq_sb, bq_sb, qpT[b]),
            ):
                xts = []
                for kt in range(KT):
                    xt = xt_pool.tile([128, L], BF16, tag="xt")
                    nc.sync.dma_start(
                        xt[:], xsrc[kt * 128:(kt + 1) * 128, b * L:(b + 1) * L]
                    )
                    xts.append(xt)
                for n in range(NT):
                    ps = psum_pp.tile([128, 512], F32, tag="pp")
                    for kt in range(KT):
                        nc.tensor.matmul(
                            ps[:],
                            lhsT=w_sb[:, kt, :],
                            rhs=xts[kt][:, n * 512:(n + 1) * 512],
                            start=(kt == 0),
                            stop=(kt == KT - 1),
                        )
                    if name != "v":
                        # evacuate with per-partition bias add, cast to bf16
                        nc.vector.tensor_tensor(
                            dstT[:, n * 512:(n + 1) * 512], ps[:],
                            bias[:].to_broadcast((EC, 512)),
                            mybir.AluOpType.add,
                        )
                    else:
                        vpt = vpt_pool.tile([128, 512], BF16, tag="vpt")
                        nc.vector.tensor_copy(vpt[:], ps[:])
                        for mb in range(4):
                            trp = psum_pp.tile([128, 128], BF16, tag="pp")
                            nc.tensor.transpose(
                                trp[:], vpt[:, mb * 128:(mb + 1) * 128], ident[:]
                            )
                            st_idx = n * 4 + mb
                            nc.vector.tensor_copy(
                                vp[b][:, st_idx, 0:D], trp[:, 0:D]
                            )
                            nc.vector.tensor_copy(
                                vp[b][:, st_idx, DP1:DP1 + D], trp[:, D:2 * D]
                            )

        def attention(b):
            for lt in range(NT):
                po = []
                for h in range(HC):
                    p = psum_pv.tile([128, 512], F32, tag="pv", name=f"po{h}")
                    po.append(p)
                for st in range(ST):
                    ps = psum_sc.tile([128, 1024], F32, tag="sc")
                    for h in range(HC):
                        nc.tensor.matmul(
                            ps[:, h * 512:(h + 1) * 512],
                            lhsT=kpT[b][h * D:(h + 1) * D, st * 128:(st + 1) * 128],
                            rhs=qpT[b][h * D:(h + 1) * D, lt * 512:(lt + 1) * 512],
                            start=True,
                            stop=True,
                            tile_position=(h * D, 0),
                        )
                    ex = exp_pool.tile([128, 1024], BF16, tag="exp")
                    nc.scalar.activation(ex[:], ps[:], AF.Exp)
                    for h in range(HC):
                        nc.tensor.matmul(
                            po[h][0:DP1, :],
                            lhsT=vp[b][:, st, h * DP1:(h + 1) * DP1],
                            rhs=ex[:, h * 512:(h + 1) * 512],
                            start=(st == 0),
                            stop=(st == ST - 1),
                        )
                for h in range(HC):
                    rc = rc_pool.tile([1, 512], F32, tag="rc")
                    rc16 = rc_pool.tile([1, 512], BF16, tag="rc16")
                    nc.vector.reciprocal(rc[:], po[h][D:DP1, :])
                    nc.vector.tensor_copy(rc16[:], rc[:])
                    pb = psum_pp.tile([128, 512], F32, tag="pp", name="pb")
                    nc.tensor.matmul(
                        pb[0:D, :], lhsT=ones_sb[:], rhs=rc16[:],
                        start=True, stop=True,
                    )
                    pb_sb = ot_pool.tile([D, 512], BF16, tag="pbsb")
                    nc.vector.tensor_copy(pb_sb[:], pb[0:D, :])
                    ot = ot_pool.tile([D, 512], BF16, tag="ot")
                    nc.vector.tensor_mul(ot[:], po[h][0:D, :], pb_sb[:])
                    nc.gpsimd.dma_start(
                        outhT[b][h * D:(h + 1) * D, lt * 512:(lt + 1) * 512], ot[:]
                    )

        def gather(b):
            nc.gpsimd.collective_compute(
                "AllGather",
                mybir.AluOpType.bypass,
                replica_groups=[list(range(NCORES))],
                ins=[outhT[b][:]],
                outs=[ag[b][:]],
            )

        def project_out(b):
            ag_sb = ag_pool.tile([128, KT, L], BF16, tag="ag")
            nc.sync.dma_start(
                ag_sb[:], ag[b][:].rearrange("(c p) r -> p c r", p=128)
            )
            for n in range(NT):
                pt = psum_pp.tile([128, 512], F32, tag="pp")
                for c2 in range(KT):
                    nc.tensor.matmul(
                        pt[:],
                        lhsT=wo_sb[:, c2, :],
                        rhs=ag_sb[:, c2, n * 512:(n + 1) * 512],
                        start=(c2 == 0),
                        stop=(c2 == KT - 1),
                    )
                ov = ov_pool.tile([128, 512], F32, tag="ov")
                nc.vector.tensor_copy(ov[:], pt[:])
                nc.sync.dma_start(
                    outT[:, b * L + n * 512: b * L + (n + 1) * 512], ov[:]
                )

        project(0)
        attention(0)
        gather(0)
        project(1)
        project_out(0)
        attention(1)
        gather(1)
        project_out(1)

    nc.compile()
    return nc


_NC_CACHE = {}


def _get_nc():
    if "nc" not in _NC_CACHE:
        _NC_CACHE["nc"] = build_nc()
    return _NC_CACHE["nc"]


def kernel(q, k, v, Wq, bq, Wk, bk, Wv, bv, Wo, bo, _trace=False, _tmpdir=None):
    bf = ml_dtypes.bfloat16
    scale = np.float32(1.0 / np.sqrt(D))  # 0.125, exact

    q2 = np.asarray(q, np.float32).reshape(R, E)
    k2 = np.asarray(k, np.float32).reshape(R, E)
    v2 = np.asarray(v, np.float32).reshape(R, E)
    qTh = np.ascontiguousarray(q2.T).astype(bf)
    kTh = np.ascontiguousarray(k2.T).astype(bf)
    vTh = np.ascontiguousarray(v2.T).astype(bf)
    Wq = np.asarray(Wq, np.float32)
    Wk = np.asarray(Wk, np.float32)
    Wv = np.asarray(Wv, np.float32)
    Wo = np.asarray(Wo, np.float32)

    in_maps = []
    for c in range(NCORES):
        sl = slice(c * EC, (c + 1) * EC)
        in_maps.append({
            "qT": qTh,
            "kT": kTh,
            "vT": vTh,
            "wq": np.ascontiguousarray(Wq[:, sl] * scale).astype(bf),
            "wk": np.ascontiguousarray(Wk[:, sl]).astype(bf),
            "wv": np.ascontiguousarray(Wv[:, sl]).astype(bf),
            "wo": np.ascontiguousarray(Wo[:, sl]).astype(bf),
            "bq": (np.asarray(bq, np.float32)[sl] * scale).reshape(EC, 1).copy(),
            "bk": np.asarray(bk, np.float32)[sl].reshape(EC, 1).copy(),
        })

    nc = _get_nc()
    res = run_bass_kernel_spmd(
        nc, in_maps, list(range(NCORES)), trace=_trace, tmpdir=_tmpdir
    )
    outT_full = np.concatenate(
        [np.asarray(res.results[c]["outT"], np.float32) for c in range(NCORES)],
        axis=0,
    )  # [E, R]
    out = np.ascontiguousarray(outT_full.T)  # [R, E]
    # bv passes through attention unchanged (softmax rows sum to 1):
    # out += bv @ Wo + bo
    host_bias = (
        np.asarray(bv, np.float64) @ np.asarray(Wo, np.float64)
        + np.asarray(bo, np.float64)
    ).astype(np.float32)
    out += host_bias[None, :]
    if _trace:
        return out.reshape(B, L, E), res
    return out.reshape(B, L, E)


# revision 10
# speedup vs baseline: 1.1731x; 1.1312x over previous
"""Trainium2 Bass kernel for nn_Attention_86646670230179 (eager MHA, f32 I/O).

Strategy (8 NeuronCores, tensor-parallel over heads):
  - Each core owns 2 of the 16 heads (a 128-wide slice of the internal dim).
  - Host stages q/k/v transposed ([E, B*L], bf16) so every matmul contracts
    along the SBUF partition dim. Score scale (1/8) is folded into Wq.
  - Per core: qp^T/kp^T projections (transposed layout), vp projection
    (natural layout via on-chip PE transpose of vp^T), scores^T = kh^T.T @ qh^T
    with both heads packed into the 128-row PE array (K=64 each),
    exp on ScalarE (no max subtraction: scores ~ N(0,1)), PV matmul with an
    appended ones-column producing unnormalized outputs + row sums in one
    accumulation, normalization via a K=1 broadcast matmul + DVE multiply.
  - Per-head outputs (outh^T, bf16) are AllGathered (one collective per batch)
    and every core computes a 128-wide column slice of the output projection
    in transposed orientation (out^T = Wo_c^T @ outh^T, N=512 matmuls).
  - Host concatenates the 8 column slices, transposes, and adds the bias
    terms (bv @ Wo + bo), which commute with attention exactly because
    softmax rows sum to 1.
"""
import sys
from contextlib import ExitStack

import numpy as np

sys.path.insert(0, "/opt/trn_rl_repo")

import ml_dtypes  # noqa: E402
import concourse.bass as bass  # noqa: E402
import concourse.mybir as mybir  # noqa: E402
import concourse.tile as tile  # noqa: E402
from concourse import bacc  # noqa: E402
from concourse.bass_utils import run_bass_kernel_spmd  # noqa: E402
from concourse.masks import make_identity  # noqa: E402

BF16 = mybir.dt.bfloat16
F32 = mybir.dt.float32
AF = mybir.ActivationFunctionType

NCORES = 8
B, L, E, H = 2, 2048, 1024, 16
S = L
D = E // H            # 64 head dim
R = B * L             # 4096 total rows
HC = H // NCORES      # 2 heads per core
EC = HC * D           # 128 channel slice per core
KT = E // 128         # 8 contraction tiles
NT = L // 512         # 4 512-wide row tiles per batch
ST = S // 128         # 16 key tiles per batch
DP1 = D + 1           # 65: head dim + ones column


def build_nc():
    nc = bacc.Bacc("TRN2", target_bir_lowering=False, num_devices=NCORES)

    qT = nc.declare_dram_parameter("qT", [E, R], BF16, isOutput=False)
    kT = nc.declare_dram_parameter("kT", [E, R], BF16, isOutput=False)
    vT = nc.declare_dram_parameter("vT", [E, R], BF16, isOutput=False)
    wq = nc.declare_dram_parameter("wq", [E, EC], BF16, isOutput=False)
    wk = nc.declare_dram_parameter("wk", [E, EC], BF16, isOutput=False)
    wv = nc.declare_dram_parameter("wv", [E, EC], BF16, isOutput=False)
    wo = nc.declare_dram_parameter("wo", [E, EC], BF16, isOutput=False)
    bq = nc.declare_dram_parameter("bq", [EC, 1], F32, isOutput=False)
    bk = nc.declare_dram_parameter("bk", [EC, 1], F32, isOutput=False)
    outT = nc.declare_dram_parameter("outT", [EC, R], F32, isOutput=True)

    with tile.TileContext(nc) as tc, ExitStack() as ctx:
        dram = ctx.enter_context(tc.tile_pool(name="dram", bufs=1, space="DRAM"))
        consts = ctx.enter_context(tc.tile_pool(name="consts", bufs=1))
        xt_pool = ctx.enter_context(tc.tile_pool(name="xt", bufs=9))
        vpt_pool = ctx.enter_context(tc.tile_pool(name="vpt", bufs=2))
        exp_pool = ctx.enter_context(tc.tile_pool(name="expp", bufs=3))
        ot_pool = ctx.enter_context(tc.tile_pool(name="otp", bufs=3))
        ag_pool = ctx.enter_context(tc.tile_pool(name="agp", bufs=2))
        ov_pool = ctx.enter_context(tc.tile_pool(name="ovp", bufs=2))
        rc_pool = ctx.enter_context(tc.tile_pool(name="rcp", bufs=3))
        # PSUM: sc 2x[128,1024] (4 banks) + pv 3x[128,512] (3) + pp 1x[128,512] (1)
        psum_sc = ctx.enter_context(tc.tile_pool(name="psc", bufs=2, space="PSUM"))
        psum_pv = ctx.enter_context(tc.tile_pool(name="ppv", bufs=2, space="PSUM"))
        psum_pp = ctx.enter_context(tc.tile_pool(name="ppp", bufs=2, space="PSUM"))

        # ---- constants / weights staging
        wq_sb = consts.tile([128, KT, EC], BF16, tag="wq")
        wk_sb = consts.tile([128, KT, EC], BF16, tag="wk")
        wv_sb = consts.tile([128, KT, EC], BF16, tag="wv")
        wo_sb = consts.tile([128, KT, EC], BF16, tag="wo")
        for w_sb, w in ((wq_sb, wq), (wk_sb, wk), (wv_sb, wv), (wo_sb, wo)):
            nc.sync.dma_start(w_sb[:], w[:].rearrange("(ko p) m -> p ko m", p=128))
        bq_sb = consts.tile([EC, 1], F32, tag="bq")
        bk_sb = consts.tile([EC, 1], F32, tag="bk")
        nc.sync.dma_start(bq_sb[:], bq[:])
        nc.sync.dma_start(bk_sb[:], bk[:])
        ones_sb = consts.tile([1, D], BF16, tag="ones")
        nc.vector.memset(ones_sb[:], 1.0)
        ident = consts.tile([128, 128], BF16, tag="ident")
        make_identity(nc, ident[:])

        # persistent per-batch activations
        qpT = [consts.tile([128, L], BF16, tag=f"qpT{b}", name=f"qpT{b}") for b in range(B)]
        kpT = [consts.tile([128, L], BF16, tag=f"kpT{b}", name=f"kpT{b}") for b in range(B)]
        vp = [consts.tile([128, ST, 2 * DP1], BF16, tag=f"vp{b}", name=f"vp{b}") for b in range(B)]
        for b in range(B):
            nc.vector.memset(vp[b][:, :, D], 1.0)
            nc.vector.memset(vp[b][:, :, 2 * D + 1], 1.0)

        outhT = [dram.tile([EC, L], BF16, tag=f"outhT{b}", name=f"outhT{b}") for b in range(B)]
        ag = [dram.tile([EC * NCORES, L], BF16, tag=f"ag{b}", name=f"ag{b}", addr_space="Shared") for b in range(B)]

        def project(b):
            """qp^T, kp^T (transposed) and vp (natural) for batch b."""
            for name, xsrc, w_sb, bias, dstT in (
                ("k", kT, wk_sb, bk_sb, kpT[b]),
                ("v", vT, wv_sb, None, None),
                ("q", qT, wq_sb, bq_sb, qpT[b]),
            ):
                xts = []
                for kt in range(KT):
                    xt = xt_pool.tile([128, L], BF16, tag="xt")
                    nc.sync.dma_start(
                        xt[:], xsrc[kt * 128:(kt + 1) * 128, b * L:(b + 1) * L]
                    )
                    xts.append(xt)
                for n in range(NT):
                    ps = psum_pp.tile([128, 512], F32, tag="pp")
                    for kt in range(KT):
                        nc.tensor.matmul(
                            ps[:],
                            lhsT=w_sb[:, kt, :],
                            rhs=xts[kt][:, n * 512:(n + 1) * 512],
                            start=(kt == 0),
                            stop=(kt == KT - 1),
                        )
                    if name != "v":
                        # evacuate with per-partition bias add, cast to bf16
                        nc.vector.tensor_tensor(
                            dstT[:, n * 512:(n + 1) * 512], ps[:],
                            bias[:].to_broadcast((EC, 512)),
                            mybir.AluOpType.add,
                        )
                    else:
                        vpt = vpt_pool.tile([128, 512], BF16, tag="vpt")
                        nc.vector.tensor_copy(vpt[:], ps[:])
                        for mb in range(4):
                            trp = psum_pp.tile([128, 128], BF16, tag="pp")
                            nc.tensor.transpose(
                                trp[:], vpt[:, mb * 128:(mb + 1) * 128], ident[:]
                            )
                            st_idx = n * 4 + mb
                            nc.vector.tensor_copy(
                                vp[b][:, st_idx, 0:D], trp[:, 0:D]
                            )
                            nc.vector.tensor_copy(
                                vp[b][:, st_idx, DP1:DP1 + D], trp[:, D:2 * D]
                            )

        def attention(b):
            for lt in range(NT):
                po = []
                for h in range(HC):
                    p = psum_pv.tile([128, 512], F32, tag="pv", name=f"po{h}")
                    po.append(p)
                for st in range(ST):
                    ps = psum_sc.tile([128, 1024], F32, tag="sc")
                    for h in range(HC):
                        nc.tensor.matmul(
                            ps[:, h * 512:(h + 1) * 512],
                            lhsT=kpT[b][h * D:(h + 1) * D, st * 128:(st + 1) * 128],
                            rhs=qpT[b][h * D:(h + 1) * D, lt * 512:(lt + 1) * 512],
                            start=True,
                            stop=True,
                            tile_position=(h * D, 0),
                        )
                    ex = exp_pool.tile([128, 1024], BF16, tag="exp")
                    nc.scalar.activation(ex[:], ps[:], AF.Exp)
                    for h in range(HC):
                        nc.tensor.matmul(
                            po[h][0:DP1, :],
                            lhsT=vp[b][:, st, h * DP1:(h + 1) * DP1],
                            rhs=ex[:, h * 512:(h + 1) * 512],
                            start=(st == 0),
                            stop=(st == ST - 1),
                        )
                for h in range(HC):
                    rc = rc_pool.tile([1, 512], F32, tag="rc")
                    rc16 = rc_pool.tile([1, 512], BF16, tag="rc16")
                    nc.vector.reciprocal(rc[:], po[h][D:DP1, :])
                    nc.vector.tensor_copy(rc16[:], rc[:])
                    pb = psum_sc.tile([128, 512], F32, tag="sc", name="pb")
                    nc.tensor.matmul(
                        pb[0:D, :], lhsT=ones_sb[:], rhs=rc16[:],
                        start=True, stop=True,
                    )
                    pb_sb = ot_pool.tile([D, 512], BF16, tag="pbsb")
                    nc.vector.tensor_copy(pb_sb[:], pb[0:D, :])
                    ot = ot_pool.tile([D, 512], BF16, tag="ot")
                    nc.vector.tensor_mul(ot[:], po[h][0:D, :], pb_sb[:])
                    nc.gpsimd.dma_start(
                        outhT[b][h * D:(h + 1) * D, lt * 512:(lt + 1) * 512], ot[:]
                    )

        def gather(b):
            nc.gpsimd.collective_compute(
                "AllGather",
                mybir.AluOpType.bypass,
                replica_groups=[list(range(NCORES))],
                ins=[outhT[b][:]],
                outs=[ag[b][:]],
            )

        def project_out(b):
            ag_sb = ag_pool.tile([128, KT, L], BF16, tag="ag")
            nc.sync.dma_start(
                ag_sb[:], ag[b][:].rearrange("(c p) r -> p c r", p=128)
            )
            for n in range(NT):
                pt = psum_pp.tile([128, 512], F32, tag="pp")
                for c2 in range(KT):
                    nc.tensor.matmul(
                        pt[:],
                        lhsT=wo_sb[:, c2, :],
                        rhs=ag_sb[:, c2, n * 512:(n + 1) * 512],
                        start=(c2 == 0),
                        stop=(c2 == KT - 1),
                    )
                ov = ov_pool.tile([128, 512], F32, tag="ov")
                nc.vector.tensor_copy(ov[:], pt[:])
                nc.sync.dma_start(
                    outT[:, b * L + n * 512: b * L + (n + 1) * 512], ov[:]
                )

        project(0)
        attention(0)
        gather(0)
        project(1)
        project_out(0)
        attention(1)
        gather(1)
        project_out(1)

    nc.compile()
    return nc


_NC_CACHE = {}


def _get_nc():
    if "nc" not in _NC_CACHE:
        _NC_CACHE["nc"] = build_nc()
    return _NC_CACHE["nc"]


def kernel(q, k, v, Wq, bq, Wk, bk, Wv, bv, Wo, bo, _trace=False, _tmpdir=None):
    bf = ml_dtypes.bfloat16
    scale = np.float32(1.0 / np.sqrt(D))  # 0.125, exact

    q2 = np.asarray(q, np.float32).reshape(R, E)
    k2 = np.asarray(k, np.float32).reshape(R, E)
    v2 = np.asarray(v, np.float32).reshape(R, E)
    qTh = np.ascontiguousarray(q2.T).astype(bf)
    kTh = np.ascontiguousarray(k2.T).astype(bf)
    vTh = np.ascontiguousarray(v2.T).astype(bf)
    Wq = np.asarray(Wq, np.float32)
    Wk = np.asarray(Wk, np.float32)
    Wv = np.asarray(Wv, np.float32)
    Wo = np.asarray(Wo, np.float32)

    in_maps = []
    for c in range(NCORES):
        sl = slice(c * EC, (c + 1) * EC)
        in_maps.append({
            "qT": qTh,
            "kT": kTh,
            "vT": vTh,
            "wq": np.ascontiguousarray(Wq[:, sl] * scale).astype(bf),
            "wk": np.ascontiguousarray(Wk[:, sl]).astype(bf),
            "wv": np.ascontiguousarray(Wv[:, sl]).astype(bf),
            "wo": np.ascontiguousarray(Wo[:, sl]).astype(bf),
            "bq": (np.asarray(bq, np.float32)[sl] * scale).reshape(EC, 1).copy(),
            "bk": np.asarray(bk, np.float32)[sl].reshape(EC, 1).copy(),
        })

    nc = _get_nc()
    res = run_bass_kernel_spmd(
        nc, in_maps, list(range(NCORES)), trace=_trace, tmpdir=_tmpdir
    )
    outT_full = np.concatenate(
        [np.asarray(res.results[c]["outT"], np.float32) for c in range(NCORES)],
        axis=0,
    )  # [E, R]
    out = np.ascontiguousarray(outT_full.T)  # [R, E]
    # bv passes through attention unchanged (softmax rows sum to 1):
    # out += bv @ Wo + bo
    host_bias = (
        np.asarray(bv, np.float64) @ np.asarray(Wo, np.float64)
        + np.asarray(bo, np.float64)
    ).astype(np.float32)
    out += host_bias[None, :]
    if _trace:
        return out.reshape(B, L, E), res
    return out.reshape(B, L, E)
